# revision 73
# baseline (speedup 1.0000x reference)
"""MoE (dense routing) Trainium2 kernel: 8-core data-parallel over tokens.

Problem: nn_MixtureOfExperts_33011118637071
  N=16384 tokens, D=256 model dim, E=8 experts, H=128 gate hidden.
  gate   = softmax(relu(x @ Wg1 + bg1) @ Wg2 + bg2)          [N, E]
  h_e    = relu(x @ W1[e] + b1[e])                           [N, D]
  y      = sum_e gate[:, e] * (h_e @ W2[e] + b2[e])          [N, D]

Strategy (per core, 2048 tokens):
  Feature-major layout (features on partitions, tokens on the free dim) so
  the two expert GEMMs chain without transposes; x is transposed on the
  host as part of sharding and the output transposed back on gather.

  v6 over v2 (92.5us -> ~66us under the TRN2 cost model):
  - All matmul operands are bf16 (same PE rate as float32r at free>=256,
    half the HBM traffic; rel err ~4e-3, well under the 2e-2 gate).
  - The gate-row broadcasts, which v2 did as K<=8 PE matmuls packed with
    tile_position, move off the PE entirely onto the DMA engines via a
    DRAM bounce: per tile, the 8 raw exp rows + the 1/sum row are written
    to a scratch DRAM buffer (tiny), then one DMA with a zero-stride
    source reads them back replicated to all 128 partitions as
    gall[128, 9, T] (~3.3us of DMA, fully off the PE). GPSIMD
    partition_broadcast would be cheaper but does not survive walrus
    codegen ("ISA wrong length"), and SBUF-source DMAs reject zero
    partition stride — DRAM-source broadcast is the one path that
    compiles. The PE then only runs real GEMM passes: per tile 64 expert
    matmuls + 2 pg1 + pg2 + exp-sum + 2 b2-init = 70 passes x 512 rows
    ~ 14.9us/tile.
  - Softmax normalization is deferred to the very end: experts accumulate
    exp-weighted (unnormalized) outputs in PSUM — the b2 term rides along
    as sum_e exp_e*b2_e — and the output evacuation multiplies by the
    replicated 1/sum row on DVE. An earlier variant that normalized the
    gate rows in SBUF head-of-line blocked the in-order DVE queue behind
    the broadcast backlog.
  - Gate-multiply on DVE reads bf16 SBUF operands only -> 2x DVE mode
    (327ns vs 658ns for the v2 PSUM-operand form).
  - The gate stages are software-pipelined across token tiles
    (pg1/relu | pg2/exp | sum/recip/bounce) so the PE streams gate
    matmuls back-to-back instead of waiting on each tile's serial
    ScalarE->PE->ScalarE chain.
  - Expert layer-2 emission lags layer-1 by one expert (l1(e+1) issues
    before l2(e)) so the relu->gate-multiply chain of expert e hides
    under expert e+1's layer-1 passes; within l2 the kc=1 operands (whose
    relu finishes last) are consumed after both kc=0 passes; the b2 PSUM
    init sits at e==1, giving the previous tile's output evacuations
    ~1.7us to free the banks.
  - W1/W2/x are host-packed into their exact SBUF layouts (trivial DMA
    descriptors). x + bounce traffic rides the SP queue in compute-need
    order; the rep-0 weight stream rides the Pool/SWDGE queue whose
    generation naturally paces it; output stores also ride the Pool queue
    to keep the Activation SEQ free of its 667ns per-DMA issue cost.
  - DMA instruction COUNT and QUEUE matter on the measured backend
    (~0.5-1us of unmodeled per-DMA overhead, SWDGE generation charged
    extra): x loads are 2 DMAs per rep (tile 0 + rest), each tile's
    output is a single [128, KC, T] store (4/rep) issued from the
    Activation/HWDGE queue — moving the stores off the Pool/SWDGE queue
    measured -4.4us. Moving the 1/sum replication to a K=1 PE pass
    measured WORSE (quad-bank pressure), as did putting the big
    replicated bounce reads on the Activation ring (head-of-line behind
    the waiting entry).
  - PSUM: 3 gate banks + 3 hidden banks + 2 output-accum banks = 8.
"""
import numpy as np
import ml_dtypes

import bass_rust
import concourse.bass as bass
import concourse.mybir as mybir
import concourse.tile as tile
from concourse.bass_utils import run_bass_kernel_spmd

F32 = mybir.dt.float32
BF16 = mybir.dt.bfloat16
AF = mybir.ActivationFunctionType

N, D, E, H = 16384, 256, 8, 128
NCORES = 8
TPC = N // NCORES          # tokens per core
T = 512                    # token tile (max fp32 PSUM moving free dim)
NT = TPC // T              # token tiles per core
KC = D // 128              # 128-row chunks of the model dim

# bf16 matmul-const tensor column layout ([128, GB_W])
GB_WG1 = 0                 # 256 cols: Wg1 as [p, kc*H + h]
GB_WG2 = 256               # 128 cols: Wg2 replicated into strips 32s+(0..7)
GB_OND = 384               # 1 col: exp-sum selector; rows 32s+(0..7) are 1,
                           # so each strip's matmul sums its 8 exp rows into
                           # output partition 0
GB_B2 = 385                # 256 cols: b2 128-col blocks at strips 0/1
GB_W = 641

# f32 bias tensor column layout ([128, GF_W])
GF_BG1 = 0                 # bg1
GF_BG2 = 1                 # bg2 replicated into strips
GF_B1 = 2                  # 16 cols: b1 as [p, e*KC + kc]
GF_W = 18

_CTR = [0]


def _split_multi_waits(nc, max_waits=1):
    """This container's walrus rejects >1 sync-wait per instruction; hoist
    extras onto fresh same-engine NoOps placed just before the waiter."""
    for fn in nc.m.functions:
        for bb in fn.blocks:
            out = []
            for inst in bb.instructions:
                si = inst.sync_info
                waits = list(si.on_wait) if si is not None and si.on_wait else []
                if len(waits) > max_waits:
                    for w in waits[:-max_waits]:
                        _CTR[0] += 1
                        nop = bass_rust.InstNoOp(
                            name=f"I-waitfix-{_CTR[0]}", ins=[], outs=[])
                        nop.engine = inst.engine
                        nop.sync_info = mybir.SyncInfo(on_wait=[w], on_update=[])
                        nc.register_instruction(nop)
                        out.append(nop)
                    si.on_wait = waits[-max_waits:]
                out.append(inst)
            bb.instructions = out


def build_nc(repeat: int = 1):
    nc = bass.Bass("TRN2", target_bir_lowering=False, debug=False,
                   num_devices=NCORES)

    # all big tensors host-packed into their SBUF layouts (trivial DMAs)
    x_d = nc.dram_tensor("xs", [128, KC, TPC], BF16, kind="ExternalInput")
    gb_d = nc.dram_tensor("gb", [128, GB_W], BF16, kind="ExternalInput")
    gf_d = nc.dram_tensor("gf", [128, GF_W], F32, kind="ExternalInput")
    W1_d = nc.dram_tensor("W1", [128, E, KC, D], BF16, kind="ExternalInput")
    W2_d = nc.dram_tensor("W2", [128, E, KC, D], BF16, kind="ExternalInput")
    yT_d = nc.dram_tensor("yT", [D, TPC], BF16, kind="ExternalOutput")
    # per-tile DRAM scratch for the gate-row broadcast bounce
    gd_d = [nc.dram_tensor(f"gd{ti}", [E + 1, T], BF16, kind="Internal")
            for ti in range(NT)]

    with tile.TileContext(nc) as tc:
        with (
            nc.allow_low_precision(reason="bf16 matmul operands"),
            tc.tile_pool(name="wpool", bufs=1) as wp,
            tc.tile_pool(name="work", bufs=3) as sb,
            tc.tile_pool(name="gbuf", bufs=NT + 1) as gb,
            tc.tile_pool(name="hbuf", bufs=4) as hb,
            tc.tile_pool(name="obuf", bufs=4) as ob,
            tc.tile_pool(name="xpool", bufs=2) as xp,
            tc.tile_pool(name="gall", bufs=NT + 1) as ga,
            tc.tile_pool(name="quad", bufs=3, space="PSUM") as quad,
            tc.tile_pool(name="phid", bufs=3, space="PSUM") as phid,
            tc.tile_pool(name="pout", bufs=2, space="PSUM") as pout,
        ):
            w1 = wp.tile([128, E, KC, D], BF16, tag="w1")
            w2 = wp.tile([128, E, KC, D], BF16, tag="w2")

            gbx = wp.tile([128, GB_W], BF16, tag="gb")
            gfx = wp.tile([128, GF_W], F32, tag="gf")
            nc.scalar.dma_start(gbx[:, :], gb_d[:, :])
            nc.scalar.dma_start(gfx[:, :], gf_d[:, :])

            def load_weights(es):
                # per-expert transfers on the Pool/SWDGE queue: descriptor
                # generation paces them ~1us apart from t~0, so they drip
                # into the serial DMA resource in need order without
                # front-running the gate-phase x/bounce traffic on SP
                for e in es:
                    nc.gpsimd.dma_start(w1[:, e, :, :], W1_d[:, e, :, :])
                    nc.gpsimd.dma_start(w2[:, e, :, :], W2_d[:, e, :, :])

            def wg1_ap(kc):
                return gbx[:, GB_WG1 + kc * H:GB_WG1 + (kc + 1) * H]
            wg2r = gbx[:, GB_WG2:GB_WG2 + 128]
            bg1 = gfx[:, GF_BG1:GF_BG1 + 1]
            bg2r = gfx[:, GF_BG2:GF_BG2 + 1]

            def ond_ap(ti):
                return gbx[32 * ti:32 * ti + 8, GB_OND:GB_OND + 1]

            def b2blk(mc):
                return gbx[32 * mc:32 * mc + 8,
                           GB_B2 + 128 * mc:GB_B2 + 128 * (mc + 1)]

            def b1bias(e, mc):
                c = GF_B1 + e * KC + mc
                return gfx[:, c:c + 1]

            def gate_a(xt, ti, rep):
                """pg1 matmuls + relu -> gate hidden rh."""
                tok = slice(ti * T, (ti + 1) * T)
                pg1 = quad.tile([128, T], F32, tag="q", name=f"pg1_{rep}_{ti}")
                for kc in range(KC):
                    nc.tensor.matmul(pg1[:, :], wg1_ap(kc), xt[:, kc, tok],
                                     start=(kc == 0), stop=(kc == KC - 1))
                rh = sb.tile([H, T], BF16, tag="rh", name=f"rh_{rep}_{ti}")
                nc.scalar.activation(rh[:, :], pg1[:, :], AF.Relu, bias=bg1)
                return rh

            def gate_b(rh, ti, rep):
                """pg2 matmul + exp -> replicated raw-exp rows expl."""
                pg2 = quad.tile([128, T], F32, tag="q", name=f"pg2_{rep}_{ti}")
                nc.tensor.matmul(pg2[:, :], wg2r, rh[:, :],
                                 start=True, stop=True)
                expl = gb.tile([128, T], BF16, tag="expl",
                               name=f"expl_{rep}_{ti}")
                nc.scalar.activation(expl[:, :], pg2[:, :], AF.Exp, bias=bg2r)
                return expl

            def gate_c(expl, ti, rep):
                """exp-sum matmul (strip ti -> output partition 0),
                reciprocal, then the broadcast bounce: raw exp rows + the
                1/sum row go to DRAM and come back replicated to all 128
                partitions as gall[128, E+1, T]."""
                qs = quad.tile([128, T], F32, tag="q", name=f"qs_{rep}_{ti}")
                nc.tensor.matmul(qs[0:1, :], ond_ap(ti),
                                 expl[32 * ti:32 * ti + 8, :],
                                 start=True, stop=True,
                                 tile_position=(32 * ti, 0))
                invr = sb.tile([1, T], BF16, tag="invr",
                               name=f"invr_{rep}_{ti}")
                nc.vector.reciprocal(invr[0:1, :], qs[0:1, :])
                # DRAM tensors are not dependency-tracked by the tile
                # framework: chain every gd access (sync=true) so the
                # replicated reads follow the row writes (RAW) and the next
                # rep's row writes follow this rep's reads (WAR)
                key = f"gd{ti}"

                def chain(inst):
                    tc.chain_iter_dep(key, inst.ins)

                chain(nc.sync.dma_start(gd_d[ti][0:E, :], expl[0:E, :]))
                chain(nc.sync.dma_start(gd_d[ti][E:E + 1, :], invr[0:1, :]))
                gall = ga.tile([128, E + 1, T], BF16, tag="gall",
                               name=f"gall_{rep}_{ti}")
                src = gd_d[ti].ap().unsqueeze(0)
                if rep == 0:
                    # rep 0 contends with the weight stream on the serial
                    # DMA resource: two halves let experts 0..3 unblock
                    # ~1.6us earlier than a monolithic replicated read
                    chain(nc.sync.dma_start(
                        gall[:, 0:4, :],
                        src[:, 0:4, :].broadcast_to([128, 4, T])))
                    chain(nc.sync.dma_start(
                        gall[:, 4:E + 1, :],
                        src[:, 4:E + 1, :].broadcast_to([128, E + 1 - 4, T])))
                else:
                    chain(nc.sync.dma_start(
                        gall[:, :, :], src.broadcast_to([128, E + 1, T])))
                return gall

            def experts_compute(xt, ti, rep, expl, gall):
                """Layer-2 emission lags layer-1 by 1.5 experts (l2(e)
                issues between l1(e+2,mc0) and l1(e+2,mc1)) so each
                expert's relu->gate-multiply chain (~1.4us) hides under
                ~1.7us of later layer-1 passes; b2 init at e==1 gives the
                previous tile's output evacuations time to free the pout
                banks."""
                tok = slice(ti * T, (ti + 1) * T)
                py = None
                hss = [None] * E

                def emit_l2(e):
                    # kc outer: the kc=1 operand's relu finishes last, so
                    # both kc=0 passes run first and buy it ~426ns
                    for kc in range(KC):
                        for mc in range(KC):
                            nc.tensor.matmul(
                                py[mc][:, :],
                                w2[:, e, kc, mc * 128:(mc + 1) * 128],
                                hss[e][:, kc, :],
                                start=False,
                                stop=(e == E - 1 and kc == KC - 1))

                for e in range(E):
                    pt = gall[:, e, :]
                    hs = hb.tile([128, KC, T], BF16, tag="hs",
                                 name=f"hs_{rep}_{ti}_{e}")
                    hss[e] = hs
                    for mc in range(KC):
                        ph = phid.tile([128, T], F32, tag="ph",
                                       name=f"ph_{rep}_{ti}_{e}_{mc}")
                        for kc in range(KC):
                            nc.tensor.matmul(
                                ph[:, :], w1[:, e, kc, mc * 128:(mc + 1) * 128],
                                xt[:, kc, tok],
                                start=(kc == 0), stop=(kc == KC - 1))
                        nc.scalar.activation(hs[:, mc, :], ph[:, :], AF.Relu,
                                             bias=b1bias(e, mc))
                        nc.vector.tensor_mul(hs[:, mc, :], hs[:, mc, :],
                                             pt[:, :])
                        if mc == 0 and e >= 2:
                            emit_l2(e - 2)
                    if e == 1:
                        py = [pout.tile([128, T], F32, tag="py",
                                        name=f"py{mc}_{rep}_{ti}")
                              for mc in range(KC)]
                        for mc in range(KC):
                            nc.tensor.matmul(py[mc][:, :], b2blk(mc),
                                             expl[32 * mc:32 * mc + 8, :],
                                             start=True, stop=False,
                                             tile_position=(32 * mc, 0))
                emit_l2(E - 2)
                emit_l2(E - 1)
                return py

            ydst = yT_d.ap().rearrange("(mc p) t -> p mc t", p=128)

            def finalize(ti, rep, py, gall):
                tok = slice(ti * T, (ti + 1) * T)
                ot = ob.tile([128, KC, T], BF16, tag="ot",
                             name=f"ot_{rep}_{ti}")
                for mc in range(KC):
                    nc.vector.tensor_mul(ot[:, mc, :], py[mc][:, :],
                                         gall[:, E, :])
                nc.scalar.dma_start(ydst[:, :, tok], ot[:, :, :])

            load_weights(range(E))

            def emit_gates(xt, rep):
                """Software-pipelined gate for one rep: A=pg1/relu,
                B=pg2/exp, C=sum/recip/bounce; stage k of tile ti issues
                while stage k+1 of tile ti-1 is still in flight."""
                rhs = [None] * NT
                expls = [None] * NT
                galls = [None] * NT
                rhs[0] = gate_a(xt, 0, rep)
                rhs[1] = gate_a(xt, 1, rep)
                expls[0] = gate_b(rhs[0], 0, rep)
                rhs[2] = gate_a(xt, 2, rep)
                expls[1] = gate_b(rhs[1], 1, rep)
                galls[0] = gate_c(expls[0], 0, rep)
                rhs[3] = gate_a(xt, 3, rep)
                expls[2] = gate_b(rhs[2], 2, rep)
                galls[1] = gate_c(expls[1], 1, rep)
                expls[3] = gate_b(rhs[3], 3, rep)
                galls[2] = gate_c(expls[2], 2, rep)
                galls[3] = gate_c(expls[3], 3, rep)
                return expls, galls

            for rep in range(repeat):
                xt = xp.tile([128, KC, TPC], BF16, tag="xt", name=f"xt{rep}")
                # tile 0 alone (fast availability for gate t0), rest as one
                # transfer: fewer DMA instructions per rep
                nc.sync.dma_start(xt[:, :, 0:T], x_d[:, :, 0:T])
                nc.sync.dma_start(xt[:, :, T:TPC], x_d[:, :, T:TPC])
                expls, galls = emit_gates(xt, rep)
                for ti in range(NT):
                    py = experts_compute(xt, ti, rep, expls[ti], galls[ti])
                    finalize(ti, rep, py, galls[ti])

    _split_multi_waits(nc)
    return nc


_NC_CACHE = None


def _get_nc():
    global _NC_CACHE
    if _NC_CACHE is None:
        _NC_CACHE = build_nc()
    return _NC_CACHE


def make_in_maps(x, Wg1, bg1, Wg2, bg2, W1, b1, W2, b2):
    bf = ml_dtypes.bfloat16
    x = np.ascontiguousarray(np.asarray(x, dtype=np.float32))
    Wg1 = np.asarray(Wg1, np.float32)
    bg1 = np.asarray(bg1, np.float32)
    Wg2 = np.asarray(Wg2, np.float32)
    bg2 = np.asarray(bg2, np.float32)
    W1 = np.asarray(W1, np.float32)
    b1 = np.asarray(b1, np.float32)
    W2 = np.asarray(W2, np.float32)
    b2 = np.asarray(b2, np.float32)

    gcb = np.zeros((128, GB_W), np.float32)
    gcf = np.zeros((128, GF_W), np.float32)
    # Wg1 [D, H] -> [p, kc*H + h]
    gcb[:, GB_WG1:GB_WG1 + KC * H] = (
        Wg1.reshape(KC, 128, H).transpose(1, 0, 2).reshape(128, KC * H))
    # Wg2 replicated: wg2r[h, 32s+k] = Wg2[h, k]; bg2 likewise per strip
    for s in range(4):
        gcb[:, GB_WG2 + 32 * s:GB_WG2 + 32 * s + 8] = Wg2
        gcf[32 * s:32 * s + 8, GF_BG2] = bg2
    gcf[:, GF_BG1] = bg1
    for j in range(4):
        gcb[32 * j:32 * j + 8, GB_OND] = 1.0
    # b2 blocks: strip mc holds b2[:, mc*128:(mc+1)*128]
    for mc in range(KC):
        gcb[32 * mc:32 * mc + 8,
            GB_B2 + 128 * mc:GB_B2 + 128 * (mc + 1)] = b2[:, mc * 128:(mc + 1) * 128]
    # b1 as [p, e*KC + kc]
    gcf[:, GF_B1:GF_B1 + E * KC] = (
        b1.reshape(E, KC, 128).transpose(2, 0, 1).reshape(128, E * KC))

    # SBUF layouts, host-packed:
    #   x:  [N, D] -> xT [D=(kc p), N] -> [p, kc, n]
    xs = np.ascontiguousarray(
        x.T.reshape(KC, 128, N).transpose(1, 0, 2).astype(bf))
    #   W:  [E, D=(kc p), D] -> [p, e, kc, d]
    w1s = np.ascontiguousarray(
        W1.reshape(E, KC, 128, D).transpose(2, 0, 1, 3).astype(bf))
    w2s = np.ascontiguousarray(
        W2.reshape(E, KC, 128, D).transpose(2, 0, 1, 3).astype(bf))

    shared = {
        "gb": np.ascontiguousarray(gcb.astype(bf)),
        "gf": np.ascontiguousarray(gcf),
        "W1": w1s,
        "W2": w2s,
    }
    return [
        {"xs": np.ascontiguousarray(xs[:, :, c * TPC:(c + 1) * TPC]), **shared}
        for c in range(NCORES)
    ]


def gather_output(results):
    out = np.empty((N, D), np.float32)
    for c in range(NCORES):
        out[c * TPC:(c + 1) * TPC, :] = (
            np.asarray(results[c]["yT"]).astype(np.float32).T)
    return out


def kernel(x, Wg1, bg1, Wg2, bg2, W1, b1, W2, b2):
    nc = _get_nc()
    in_maps = make_in_maps(x, Wg1, bg1, Wg2, bg2, W1, b1, W2, b2)
    r = run_bass_kernel_spmd(nc, in_maps, list(range(NCORES)))
    return gather_output(r.results)
